# revision 32
# baseline (speedup 1.0000x reference)
"""Bass/Tile kernel for nn_SpaceTransformer_nat: one frame per NeuronCore.

v3: chunk-pipelined head (LN1+QKV+k/v repack per 512-token chunk), PE
warm-up stream, merged per-row softmax (one strided-AP exp + one strided
tensor_mul covering both head pairs), AV via concurrent col-group matmuls
(data cols 0-63, static-ones denominator cols 64-127; ones stationary
loaded once), first-touch start=True replaces ring-zero matmuls,
Newton-rsqrt LN stats on GpSimd (no Sqrt table loads; only exp + gelu
table sets ever load), reciprocal_approx_fast straight from PSUM for the
softmax denominator, chunk-pipelined tail with per-chunk output DMA.
"""
from contextlib import ExitStack

import numpy as np
import ml_dtypes

import concourse.bass as bass
from concourse import mybir
from concourse.masks import make_identity

F32 = mybir.dt.float32
BF16 = mybir.dt.bfloat16
AF = mybir.ActivationFunctionType
ALU = mybir.AluOpType

K = 7
NH = 4
H = W = 48
C = 128
NTOK = H * W
NT = NTOK // 128
HID = 256
QN_MAX = 10          # max query rows per key row
GRP = 10             # query rows per ring group (48 -> groups of 10,10,10,10,8)
NGRP = (H + GRP - 1) // GRP

# token-tile (128) ranges per 512-token chunk, and fully-covered key rows
NTCH = [(0, 4), (4, 8), (8, 12), (12, 16), (16, 18)]
ROWCH = [(0, 10), (10, 21), (21, 32), (32, 42), (42, 48)]


def nbr_start(h):
    return min(max(h - K // 2, 0), H - K)


def qwin(r):
    rows = [h for h in range(H) if nbr_start(h) <= r < nbr_start(h) + K]
    assert rows == list(range(rows[0], rows[0] + len(rows)))
    assert len(rows) <= QN_MAX
    return rows[0], len(rows)


def et_cls(r):
    """Dedupe class for key-row r's ET table (15 classes)."""
    if r <= 6:
        return r
    if r <= 40:
        return 7
    return r - 33


N_CLS = 15


def grp_rows(g):
    return GRP * g, min(GRP * g + GRP, H)


def ring_col(h):
    return (h // GRP % 2) * 512 + (h % GRP) * 48


def segments(qs, qn):
    """Split query rows [qs, qs+qn) at ring-group boundaries."""
    segs = []
    h = qs
    while h < qs + qn:
        h1 = min((h // GRP + 1) * GRP, qs + qn)
        segs.append((h, h1))
        h = h1
    return segs


# --------------------------------------------------------------------------
# host-side preparation
# --------------------------------------------------------------------------

def build_et2(rpb):
    """ET2 [128, N_CLS*1024] bf16: rows m*64+kw; cols c*1024 + p*512 +
    dq*48 + w = exp(rpb[2p+m, bh, bw]) masked to the valid band."""
    ws = np.array([nbr_start(w) for w in range(W)])
    kw_g = np.arange(W)
    wvalid = (kw_g[:, None] >= ws[None, :]) & (kw_g[:, None] < ws[None, :] + K)
    bw_idx = np.clip(kw_g[:, None] - np.arange(W)[None, :] + (K - 1),
                     0, 2 * K - 2)

    reps = {}
    for r in range(H):
        c = et_cls(r)
        if c not in reps:
            reps[c] = r
    ET = np.zeros((128, N_CLS * 1024), np.float32)
    for c, r in reps.items():
        qs, qn = qwin(r)
        for dq in range(qn):
            h = qs + dq
            bh = r - h + (K - 1)
            assert 0 <= bh <= 2 * K - 2
            for n in range(NH):
                p, m = divmod(n, 2)
                tab = np.where(wvalid, np.exp(rpb[n, bh])[bw_idx.ravel()]
                               .reshape(W, W), 0.0)
                ET[m * 64:m * 64 + 48,
                   c * 1024 + p * 512 + dq * 48:
                   c * 1024 + p * 512 + dq * 48 + 48] = tab
    return ET


def prep_weights(inputs):
    """Fold LN affines + q-scale into weights; pack for the kernel."""
    bf = ml_dtypes.bfloat16
    n1w, n1b = inputs["norm1_w"], inputs["norm1_b"]
    n2w, n2b = inputs["norm2_w"], inputs["norm2_b"]
    qkv_w, qkv_b = inputs["qkv_w"], inputs["qkv_b"]
    sc = (C // NH) ** -0.5

    Wqkv = qkv_w * n1w[None, :]
    bq = qkv_w @ n1b + qkv_b
    Wqkv[0:C] *= sc
    bq0 = bq.copy()
    bq0[0:C] *= sc

    W1 = inputs["fc1_w"] * n2w[None, :]
    b1 = inputs["fc1_w"] @ n2b + inputs["fc1_b"]

    d = {
        "wqkv_t": np.ascontiguousarray(Wqkv.T).astype(bf),       # [128, 384]
        "bqkv": np.ascontiguousarray(bq0.reshape(3, 128).T).astype(np.float32),  # [128, 3]
        "wproj_t": np.ascontiguousarray(inputs["proj_w"].T).astype(bf),
        "bproj": inputs["proj_b"].reshape(-1, 1).astype(np.float32),
        "wfc1_t": np.ascontiguousarray(W1.T).astype(bf),         # [128, 256]
        "bfc1": np.ascontiguousarray(b1.reshape(2, 128).T).astype(np.float32),
        # fc2: [128, 256] rows=khalf-cols packed: wfc2_p[:, j*128:...] = fc2_w.T[128j:128j+128, :]
        "wfc2_p": np.ascontiguousarray(
            np.concatenate([inputs["fc2_w"].T[0:128, :],
                            inputs["fc2_w"].T[128:256, :]], axis=1)).astype(bf),
        "bfc2": inputs["fc2_b"].reshape(-1, 1).astype(np.float32),
        "et": build_et2(inputs["rpb"]).astype(bf),
        "vt_base": build_vt_base().astype(bf),
    }
    return d


def build_vt_base():
    """Static vT2h base [128, H*256]: zeros with denominator-ones.
    Slot r (256 cols): [p0-data | ones | p1-data | ones]; partition
    m*64+kw (kw<48) has 1 at slot col p*128 + 64 + 32m + (0..31)."""
    vt = np.zeros((128, H * 256), np.float32)
    for p in range(2):
        for m in range(2):
            rows = slice(m * 64, m * 64 + 48)
            for r in range(H):
                c0 = r * 256 + p * 128 + 64 + 32 * m
                vt[rows, c0:c0 + 32] = 1.0
    return vt


# --------------------------------------------------------------------------
# kernel emission
# --------------------------------------------------------------------------

def ln_chunk(nc, sb, ps, src_cp, dst_cp, ident, eps, t0, t1, tag,
             pstag=None):
    """LN over channels for token tiles [t0, t1) (<=4). Token-major stats
    via PE transpose; rstd = abs_rsqrt(var + eps) in one ACT op."""
    nt = t1 - t0
    xT_ps = ps.tile([128, 512], BF16, tag=pstag or f"lnTp{tag}", bufs=2)
    for k in range(nt):
        s = slice((t0 + k) * 128, (t0 + k + 1) * 128)
        nc.tensor.matmul(xT_ps[:, k * 128:(k + 1) * 128],
                         src_cp[:, s], ident[:], is_transpose=True,
                         start=(k == 0), stop=(k == nt - 1),
                         skip_group_check=True)
    xTs = sb.tile([128, 512], BF16, tag=f"lnxT{tag}")
    nc.vector.tensor_copy(xTs[:, :nt * 128], xT_ps[:, :nt * 128])
    mv = sb.tile([128, 4, 2], F32, tag=f"lnmv{tag}")
    for k in range(nt):
        stats = sb.tile([128, 6], F32, tag=f"lnst{tag}")
        nc.vector.bn_stats(stats[:], xTs[:, k * 128:(k + 1) * 128])
        nc.vector.bn_aggr(mv[:, k, :], stats[:])
    y = sb.tile([128, 4], F32, tag=f"lnY{tag}")
    nc.scalar.activation(y[:, 0:nt], mv[:, 0:nt, 1],
                         AF.Abs_reciprocal_sqrt, bias=eps[:])
    # apply on ACT: u = x*rstd + (-mean*rstd), per-partition scale/bias APs
    nb = sb.tile([128, 4], F32, tag=f"lnNB{tag}")
    nc.vector.tensor_mul(nb[:, 0:nt], mv[:, 0:nt, 0], y[:, 0:nt])
    nc.vector.tensor_scalar(out=nb[:, 0:nt], in0=nb[:, 0:nt],
                            scalar1=-1.0, scalar2=None, op0=ALU.mult)
    uT = sb.tile([128, 512], BF16, tag=f"lnuT{tag}")
    for k in range(nt):
        nc.scalar.activation(
            uT[:, k * 128:(k + 1) * 128], xTs[:, k * 128:(k + 1) * 128],
            AF.Identity, bias=nb[:, k:k + 1], scale=y[:, k:k + 1])
    u_ps = ps.tile([128, 512], BF16,
                   tag=pstag or f"lnup{tag}", bufs=2 if pstag else 1)
    for k in range(nt):
        nc.tensor.matmul(u_ps[:, k * 128:(k + 1) * 128],
                         uT[:, k * 128:(k + 1) * 128], ident[:],
                         is_transpose=True,
                         start=(k == 0), stop=(k == nt - 1),
                         skip_group_check=True)
    nc.vector.tensor_copy(dst_cp[:, t0 * 128:t1 * 128], u_ps[:, :nt * 128])


def emit_frame(nc, tc, ctx: ExitStack, debug_taps=False):
    # ---- IO ----
    x_in = nc.declare_dram_parameter("x_frame", [C, NTOK], F32, isOutput=False)
    wqkv = nc.declare_dram_parameter("wqkv_t", [C, 3 * C], BF16, isOutput=False)
    bqkv = nc.declare_dram_parameter("bqkv", [C, 3], F32, isOutput=False)
    wproj = nc.declare_dram_parameter("wproj_t", [C, C], BF16, isOutput=False)
    bproj = nc.declare_dram_parameter("bproj", [C, 1], F32, isOutput=False)
    wfc1 = nc.declare_dram_parameter("wfc1_t", [C, HID], BF16, isOutput=False)
    bfc1 = nc.declare_dram_parameter("bfc1", [C, 2], F32, isOutput=False)
    wfc2 = nc.declare_dram_parameter("wfc2_p", [C, HID], BF16, isOutput=False)
    bfc2 = nc.declare_dram_parameter("bfc2", [C, 1], F32, isOutput=False)
    et_in = nc.declare_dram_parameter("et", [128, N_CLS * 1024], BF16,
                                      isOutput=False)
    vtb_in = nc.declare_dram_parameter("vt_base", [128, H * 256], BF16,
                                       isOutput=False)
    out_d = nc.declare_dram_parameter("out_frame", [C, NTOK], F32,
                                      isOutput=True)
    taps = {}
    if debug_taps:
        for nm, shp, dt in [("tap_u", [C, NTOK], BF16),
                            ("tap_qkv", [C, 3 * NTOK], BF16),
                            ("tap_k2h", [C, H * 128], BF16),
                            ("tap_vT2h", [128, H * 256], BF16),
                            ("tap_osb", [C, NTOK], BF16),
                            ("tap_den", [C, NTOK], F32),
                            ("tap_recip", [C, NTOK], F32),
                            ("tap_onorm", [C, NTOK], BF16),
                            ("tap_t", [C, NTOK], BF16),
                            ("tap_z", [C, NTOK], BF16),
                            ("tap_g", [C, 2 * NTOK], BF16)]:
            taps[nm] = nc.declare_dram_parameter(nm, shp, dt, isOutput=True)

    sb = ctx.enter_context(tc.tile_pool(name="sb", bufs=3))
    big = ctx.enter_context(tc.tile_pool(name="big", bufs=1))

    # ---- loads (x split per chunk so the head pipeline starts early) ----
    x = big.tile([C, NTOK], F32)
    for t0, t1 in NTCH:
        nc.sync.dma_start(x[:, t0 * 128:t1 * 128], x_in[:, t0 * 128:t1 * 128])
    w_qkv = big.tile([C, 3 * C], BF16)
    nc.sync.dma_start(w_qkv[:], wqkv[:])
    b_qkv = big.tile([C, 3], F32)
    nc.sync.dma_start(b_qkv[:], bqkv[:])
    w_proj = big.tile([C, C], BF16)
    nc.sync.dma_start(w_proj[:], wproj[:])
    b_proj = big.tile([C, 1], F32)
    nc.sync.dma_start(b_proj[:], bproj[:])
    w_fc1 = big.tile([C, HID], BF16)
    nc.sync.dma_start(w_fc1[:], wfc1[:])
    b_fc1 = big.tile([C, 2], F32)
    nc.sync.dma_start(b_fc1[:], bfc1[:])
    w_fc2 = big.tile([C, HID], BF16)
    nc.sync.dma_start(w_fc2[:], wfc2[:])
    b_fc2 = big.tile([C, 1], F32)
    nc.sync.dma_start(b_fc2[:], bfc2[:])
    et = big.tile([128, N_CLS * 1024], BF16)
    nc.sync.dma_start(et[:], et_in[:])

    ident = big.tile([128, 128], BF16)
    make_identity(nc, ident[:])
    eps_t = big.tile([128, 1], F32)
    nc.vector.memset(eps_t[:], 1e-5)

    # k2h: [128=(p,m,c), H*128]; col r*128 + m*64 + kw
    k2h = big.tile([128, H * 128], BF16, tag="k2h")
    # v2h: [128=(n,c) rows 32n..32n+32], col r*128 + (n%2)*64 + kw
    v2h = big.tile([128, H * 128], BF16, tag="v2h")
    # vT2h slot r (256 cols): [p0-data | ones | p1-data | ones];
    # static zeros+ones base DMAd from HBM, data cols overwritten per slot.
    vT2h = big.tile([128, H * 256], BF16, tag="vT2h")
    nc.sync.dma_start(vT2h[:], vtb_in[:])

    xb = big.tile([C, NTOK], BF16, tag="xb")
    u = big.tile([C, NTOK], BF16, tag="u")
    qkv = big.tile([128, 3 * NTOK], BF16)  # cols: [q | k | v] per NTOK
    q_cp = qkv[:, 0:NTOK]
    k_cp = qkv[:, NTOK:2 * NTOK]
    v_cp = qkv[:, 2 * NTOK:3 * NTOK]

    osb = big.tile([C, NTOK], BF16)      # unnormalized O rows (c-order)
    den_f = big.tile([C, NTOK], F32, tag="denf")    # raw softmax denominators
    onorm = big.tile([C, NTOK], BF16)

    drain_r = {}   # key row r -> list of groups final after r
    for g in range(NGRP):
        h0, h1 = grp_rows(g)
        drain_r.setdefault(nbr_start(h1 - 1) + K - 1, []).append(g)

    # ---- merged head + attention pipeline --------------------------------
    # One shared 2-deep PSUM ring (tag "st", [128,1024] slots) serves the
    # LN1 transposes, QKV matmuls, vT transposes AND the score tiles, so
    # attention rows are emitted as soon as their chunk's data is ready
    # and every engine queue interleaves head and attention work.
    with tc.tile_pool(name="sp", bufs=2, space="PSUM") as sp, \
         tc.tile_pool(name="attpsO", bufs=1, space="PSUM") as psO:
        ot0 = psO.tile([128, 1024], F32, tag="ot0")
        ot1 = psO.tile([128, 1024], F32, tag="ot1")
        ot = [ot0, ot1]
        touched = set()
        st_t = {}

        # PE warm-up: dependency-free back-to-back matmuls (~2.6us)
        for i in range(12):
            wt = sp.tile([128, 1024], F32, tag="st")
            nc.tensor.matmul(wt[:, 0:128], ident[:], ident[:],
                             start=True, stop=True, skip_group_check=True)
            nc.tensor.matmul(wt[:, 128:256], ident[:], ident[:],
                             start=True, stop=True, skip_group_check=True)

        def scores(r):
            qs, qn = qwin(r)
            nq = 48 * qn
            st = sp.tile([128, 1024], F32, tag="st")
            st_t[r] = st
            for p in range(2):
                nc.tensor.matmul(
                    st[:, p * 512:p * 512 + nq],
                    k2h[p * 64:(p + 1) * 64, r * 128:(r + 1) * 128],
                    q_cp[p * 64:(p + 1) * 64, 48 * qs:48 * qs + nq],
                    start=True, stop=True, skip_group_check=True)

        def consume(r):
            qs, qn = qwin(r)
            nq = 48 * qn
            cls = et_cls(r)
            st = st_t.pop(r)
            pexp = sb.tile([128, 1024], BF16, tag="pexp")
            pmul = sb.tile([128, 1024], BF16, tag="pmul")
            stv = st[:].rearrange("p (b c) -> p b c", b=2)[:, :, 0:nq]
            pev = pexp[:].rearrange("p (b c) -> p b c", b=2)[:, :, 0:nq]
            pmv = pmul[:].rearrange("p (b c) -> p b c", b=2)[:, :, 0:nq]
            etv = et[:, cls * 1024:(cls + 1) * 1024].rearrange(
                "p (b c) -> p b c", b=2)[:, :, 0:nq]
            nc.scalar.activation(pev, stv, AF.Exp)
            nc.vector.tensor_mul(pmv, pev, etv)
            for (h0s, h1s) in segments(qs, qn):
                rc0 = ring_col(h0s)
                wseg = 48 * (h1s - h0s)
                g = h0s // GRP
                for p in range(2):
                    pc0 = p * 512 + (h0s - qs) * 48
                    first = (p, g) not in touched
                    touched.add((p, g))
                    nc.tensor.matmul(
                        ot[p][:, rc0:rc0 + wseg],
                        vT2h[0:112,
                             r * 256 + p * 128:r * 256 + p * 128 + 128],
                        pmul[0:112, pc0:pc0 + wseg],
                        start=first, stop=False, skip_group_check=True)
            for g in drain_r.get(r, []):
                h0, h1 = grp_rows(g)
                wg = 48 * (h1 - h0)
                bc = (g % 2) * 512
                cols = slice(48 * h0, 48 * h0 + wg)
                for p in range(2):
                    nc.vector.tensor_copy(
                        osb[64 * p:64 * p + 64, cols], ot[p][0:64, bc:bc + wg])
                    nc.vector.tensor_copy(
                        den_f[64 * p:64 * p + 64, cols],
                        ot[p][64:128, bc:bc + wg])

        pos = {"scored": 0, "consumed": 0}

        def pump(upto):
            while pos["consumed"] < upto:
                while pos["scored"] <= pos["consumed"] + 1 and \
                        pos["scored"] < H:
                    scores(pos["scored"])
                    pos["scored"] += 1
                consume(pos["consumed"])
                pos["consumed"] += 1

        for c in range(5):
            t0, t1 = NTCH[c]
            r0, r1 = ROWCH[c]
            cc = slice(t0 * 128, t1 * 128)
            cw = (t1 - t0) * 128
            nc.vector.tensor_copy(xb[:, cc], x[:, cc])
            ln_chunk(nc, sb, sp, xb, u, ident, eps_t, t0, t1, "1",
                     pstag="st")
            for s in range(3):
                pt = sp.tile([128, 1024], F32, tag="st")
                nc.tensor.matmul(pt[:, :cw], w_qkv[:, s * C:(s + 1) * C],
                                 u[:, cc], start=True, stop=True)
                dst = qkv[:, s * NTOK + t0 * 128: s * NTOK + t1 * 128]
                if s == 0:
                    nc.vector.tensor_scalar_add(out=dst, in0=pt[:, :cw],
                                                scalar1=b_qkv[:, s:s + 1])
                else:
                    nc.scalar.activation(dst, pt[:, :cw], AF.Identity,
                                         bias=b_qkv[:, s:s + 1])
            nc.gpsimd.memset(k2h[:, r0 * 128:r1 * 128], 0.0)
            nc.gpsimd.memset(v2h[:, r0 * 128:r1 * 128], 0.0)
            for p in range(2):
                for m in range(2):
                    rows = slice(p * 64 + m * 32, p * 64 + m * 32 + 32)
                    dst = k2h[rows, :].rearrange("c (r g) -> c r g", g=128)[
                        :, r0:r1, m * 64:m * 64 + W]
                    src = k_cp[rows, :].rearrange("c (r w) -> c r w", r=H)[
                        :, r0:r1, :]
                    nc.sync.dma_start(dst, src)
            for n in range(NH):
                m = n % 2
                rows = slice(32 * n, 32 * n + 32)
                dst = v2h[rows, :].rearrange("c (r g) -> c r g", g=128)[
                    :, r0:r1, m * 64:m * 64 + W]
                src = v_cp[rows, :].rearrange("c (r w) -> c r w", r=H)[
                    :, r0:r1, :]
                nc.gpsimd.dma_start(dst, src)
            rows = list(range(r0, r1))
            for gi in range(0, len(rows), 4):
                grp = rows[gi:gi + 4]
                ng = len(grp)
                tps = sp.tile([128, 512], BF16, tag="st")
                for k, r in enumerate(grp):
                    nc.tensor.matmul(
                        tps[:, k * 128:(k + 1) * 128],
                        v2h[:, r * 128:(r + 1) * 128], ident[:],
                        is_transpose=True, start=(k == 0),
                        stop=(k == ng - 1), skip_group_check=True)
                tv = tps[:, :ng * 128].rearrange("c (r g) -> c r g", g=128)
                dv = vT2h[:, grp[0] * 256:(grp[-1] + 1) * 256].rearrange(
                    "c (r g) -> c r g", g=256)
                if (gi // 4) % 2 == 0:
                    nc.scalar.copy(dv[:, :, 0:64], tv[:, :, 0:64])
                    nc.vector.tensor_copy(dv[:, :, 128:192], tv[:, :, 64:128])
                else:
                    nc.vector.tensor_copy(dv[:, :, 0:64], tv[:, :, 0:64])
                    nc.scalar.copy(dv[:, :, 128:192], tv[:, :, 64:128])
            # emit attention rows whose inputs are fully repacked
            pump(48 if c == 4 else ROWCH[c][1] - 4)

    if debug_taps:
        nc.sync.dma_start(taps["tap_u"][:], u[:])
        nc.sync.dma_start(taps["tap_qkv"][:], qkv[:])
        nc.sync.dma_start(taps["tap_k2h"][:], k2h[:])
        nc.sync.dma_start(taps["tap_vT2h"][:], vT2h[:])
        nc.sync.dma_start(taps["tap_osb"][:], osb[:])
        nc.sync.dma_start(taps["tap_den"][:], den_f[:])

    # ---- normalize: one big reciprocal + multiply ------------------------
    recip_f = big.tile([C, NTOK], F32, tag="v2h")   # reuses v2h slot
    nc.vector.reciprocal_approx_fast(out=recip_f[:], in_=den_f[:])
    nc.vector.tensor_mul(onorm[:], osb[:], recip_f[:])

    if debug_taps:
        nc.sync.dma_start(taps["tap_recip"][:], recip_f[:])
        nc.sync.dma_start(taps["tap_onorm"][:], onorm[:])

    # ---- tail: proj+residual, LN2, MLP, output — all per 512-chunk ----
    mm = ctx.enter_context(tc.tile_pool(name="tailps", bufs=3, space="PSUM"))
    tb = big.tile([C, NTOK], BF16, tag="xb")
    for c in range(5):
        t0, t1 = NTCH[c]
        cc = slice(t0 * 128, t1 * 128)
        cw = (t1 - t0) * 128
        pt = mm.tile([128, 512], F32, tag="mmps")
        nc.tensor.matmul(pt[:, :cw], w_proj[:], onorm[:, cc],
                         start=True, stop=True)
        nc.vector.scalar_tensor_tensor(
            out=tb[:, cc], in0=pt[:, :cw],
            scalar=b_proj[:, 0:1], in1=x[:, cc],
            op0=ALU.add, op1=ALU.add)

    if debug_taps:
        nc.sync.dma_start(taps["tap_t"][:], tb[:])

    z = big.tile([C, NTOK], BF16, tag="z")
    for c in range(5):
        ln_chunk(nc, sb, mm, tb, z, ident, eps_t, NTCH[c][0], NTCH[c][1], "2")

    g = big.tile([128, 2 * NTOK], BF16, tag="k2h")   # reuses k2h slot
    for c in range(5):
        t0, t1 = NTCH[c]
        cc = slice(t0 * 128, t1 * 128)
        cw = (t1 - t0) * 128
        for j in range(2):
            pt = mm.tile([128, 512], F32, tag="mmps")
            nc.tensor.matmul(pt[:, :cw], w_fc1[:, j * 128:(j + 1) * 128],
                             z[:, cc], start=True, stop=True)
            nc.scalar.activation(
                g[:, j * NTOK + t0 * 128: j * NTOK + t1 * 128],
                pt[:, :cw], AF.Gelu, bias=b_fc1[:, j:j + 1])

    if debug_taps:
        nc.sync.dma_start(taps["tap_z"][:], z[:])
        nc.sync.dma_start(taps["tap_g"][:], g[:])

    out_t = big.tile([C, NTOK], F32, tag="denf")  # den_f slot is free now
    for c in range(5):
        t0, t1 = NTCH[c]
        cc = slice(t0 * 128, t1 * 128)
        cw = (t1 - t0) * 128
        pt = mm.tile([128, 512], F32, tag="mmps")
        for j in range(2):
            nc.tensor.matmul(pt[:, :cw], w_fc2[:, j * C:(j + 1) * C],
                             g[:, j * NTOK + t0 * 128: j * NTOK + t1 * 128],
                             start=(j == 0), stop=(j == 1))
        nc.vector.scalar_tensor_tensor(
            out=out_t[:, cc], in0=pt[:, :cw],
            scalar=b_fc2[:, 0:1], in1=tb[:, cc],
            op0=ALU.add, op1=ALU.add)
        nc.sync.dma_start(out_d[:, cc], out_t[:, cc])


# --------------------------------------------------------------------------
# SPMD entry point: full inputs -> full output on 8 NeuronCores
# --------------------------------------------------------------------------
import concourse.tile as _tile
import concourse.bacc as _bacc
from concourse.bass_utils import run_bass_kernel_spmd as _run_spmd

_CACHE = {}


def _get_nc():
    if "nc" not in _CACHE:
        nc = _bacc.Bacc("TRN2", target_bir_lowering=False, debug=False,
                        num_devices=8)
        with _tile.TileContext(nc) as tc:
            with ExitStack() as ctx:
                emit_frame(nc, tc, ctx)
        nc.compile()
        _CACHE["nc"] = nc
    return _CACHE["nc"]


def kernel(**inputs):
    inputs = {k: np.asarray(v) for k, v in inputs.items()}
    x = inputs["x"]
    B, Cc, D, Hh, Ww = x.shape          # (2, 128, 4, 48, 48)
    assert (B, Cc, D, Hh, Ww) == (2, 128, 4, 48, 48)
    wd = prep_weights(inputs)
    nc = _get_nc()

    in_maps = []
    for core in range(8):
        b, dd = divmod(core, D)
        frame = np.ascontiguousarray(x[b, :, dd]).reshape(C, NTOK)
        m = {"x_frame": frame.astype(np.float32)}
        m.update(wd)
        in_maps.append(m)

    res = _run_spmd(nc, in_maps, list(range(8)))
    out = np.empty((B, Cc, D, Hh, Ww), np.float32)
    for core in range(8):
        b, dd = divmod(core, D)
        out[b, :, dd] = res.results[core]["out_frame"].reshape(C, Hh, Ww)
    return out
